# revision 30
# baseline (speedup 1.0000x reference)
"""nn_MultiHeadAttention: fused MHA + residual + LayerNorm on 8 TRN2 NeuronCores.

Sharding: core = (batch b, query-half). Each core computes, for its batch:
  - Q projection for its 512 query rows, K/V projections for all 1024 keys
    (K/V work duplicated within a batch pair -> zero cross-core communication),
  - all 16 heads' attention for its query rows,
  - output projection + residual + LayerNorm for its rows.
Host concatenates the 8 [512, 1024] results into [4, 1024, 1024].

All matmuls run in float32r (TF32-like, 1 cycle/row at N=512; plain fp32 is
4x slower). Layouts are transposed so softmax sums come free as an extra
ones-column in the PV matmul:
  qT/kT = W.T @ x.T   [d_model_out, s]   (head h = partitions 64*(h%2) of chunk h//2)
  scoresT[sk, sq] = kT_h.T @ qT_h        (K=64 matmul)
  p = exp(scores/8)   (no max-subtraction: scores ~ N(0,1), exp is safe in fp32)
  xT[d(+1), sq] = [v | 1].T @ p          (row 64 = softmax denominators)

PSUM discipline: every accumulator is a [128, 1024] 2-bank tile from one
4-slot pool (exactly 8 banks). The ACT-bound attention phase interleaves
dummy "heater" matmuls so the PE's HAM clock gate stays at 2.4 GHz.
"""
import numpy as np

import concourse.bass as bass
import concourse.mybir as mybir
import concourse.tile as tile
from concourse import bacc, bass_utils

B, S, D, H, DK = 4, 1024, 1024, 16, 64
P = 128
SH = S // 2           # query rows per core
NC = D // P           # 8 chunks of 128 along any d-dimension
NCORES = 8
EPS = 1e-6
f32 = mybir.dt.float32
f32r = mybir.dt.float32r

TRACE = False          # set by test.py to profile
LAST_EXEC_NS = None

_CACHE = {}


def _build():
    nc = bacc.Bacc("TRN2")
    xqT = nc.dram_tensor("xqT", [D, SH], f32, kind="ExternalInput")
    xkT = nc.dram_tensor("xkT", [D, S], f32, kind="ExternalInput")
    xvT = nc.dram_tensor("xvT", [D, S], f32, kind="ExternalInput")
    wq = nc.dram_tensor("wq", [D, D], f32, kind="ExternalInput")   # Wq.T  [in, out]
    wk = nc.dram_tensor("wk", [D, D], f32, kind="ExternalInput")
    wv = nc.dram_tensor("wv", [D, D], f32, kind="ExternalInput")
    wo = nc.dram_tensor("wo", [D, D], f32, kind="ExternalInput")   # Wo.T  [d, e]
    resid = nc.dram_tensor("resid", [SH, D], f32, kind="ExternalInput")
    gamma = nc.dram_tensor("gamma", [D], f32, kind="ExternalInput")
    beta = nc.dram_tensor("beta", [D], f32, kind="ExternalInput")
    out = nc.dram_tensor("out", [SH, D], f32, kind="ExternalOutput")

    with tile.TileContext(nc) as tc:
        with (
            tc.tile_pool(name="wpool", bufs=9) as wpool,
            tc.tile_pool(name="xpool", bufs=7) as xpool,
            tc.tile_pool(name="persist", bufs=1) as persist,
            tc.tile_pool(name="expp", bufs=2) as expp,
            tc.tile_pool(name="small", bufs=2) as small,
            tc.tile_pool(name="lnp", bufs=2) as lnp,
            tc.tile_pool(name="psum", bufs=4, space="PSUM") as psum,
        ):
            # ---------------- persistent tiles ----------------
            qT = persist.tile([P, NC, SH], f32r)     # [j, sq]
            kT = persist.tile([P, NC, S], f32r)      # [j, sk]
            vt = persist.tile([P, NC, H, DK + 1], f32r)  # [sk, (h, d|1)]
            xT = persist.tile([P, NC, SH], f32r)     # normalized attn out, [d, sq]
            gb = persist.tile([P, 2, D], f32)        # gamma/beta broadcast
            eps_t = persist.tile([P, 1], f32)

            nc.vector.memset(eps_t, EPS)
            nc.vector.memset(vt[:, :, :, DK:DK + 1].bitcast(f32), 1.0)  # ones col

            def load_w_chunk(w, i, nm):
                """One [128, 1024] weight chunk, 8x 64KB sub-DMAs."""
                wt = wpool.tile([P, D], f32r, tag="w", name=f"{nm}{i}")
                for q in range(4):
                    nc.sync.dma_start(
                        wt[:, q * 256:(q + 1) * 256],
                        w[i * P:(i + 1) * P, q * 256:(q + 1) * 256].bitcast(f32r),
                    )
                return wt

            def load_w(w, nm):
                return [load_w_chunk(w, i, nm) for i in range(NC)]

            def load_x(x, i, col0):
                """One [128, 512] half-chunk of a transposed input, 4x 64KB sub-DMAs."""
                xc = xpool.tile([P, SH], f32r, tag="x", name="xc")
                for q in range(4):
                    nc.sync.dma_start(
                        xc[:, q * P:(q + 1) * P],
                        x[i * P:(i + 1) * P, col0 + q * P:col0 + (q + 1) * P]
                        .bitcast(f32r),
                    )
                return xc

            def ps_tile(nm):
                return psum.tile([P, 2 * SH], f32, tag="mm", name=nm)

            # ---------------- Q projection ----------------
            # qT[j, sq] = sum_i Wq.T[i, j] * xqT[i, sq]
            ps_q = [ps_tile(f"psq{t}") for t in range(4)]
            wq8 = []
            for i in range(NC):
                wq8.append(load_w_chunk(wq, i, "wq"))
                xc = load_x(xqT, i, 0)
                for j in range(NC):
                    nc.tensor.matmul(
                        ps_q[j // 2][:, (j % 2) * SH:(j % 2 + 1) * SH],
                        wq8[i][:, j * P:(j + 1) * P], xc,
                        start=(i == 0), stop=(i == NC - 1),
                    )
            for t in range(4):
                nc.scalar.copy(qT[:, 2 * t, :], ps_q[t][:, :SH])
                nc.vector.tensor_copy(qT[:, 2 * t + 1, :], ps_q[t][:, SH:])

            # ---------------- K projection ----------------
            # kT[j, sk]; 2 passes over sk-halves, each streams half of xk once
            wk8 = []
            for sh in range(2):
                ps_k = [ps_tile(f"psk{t}") for t in range(4)]
                for i in range(NC):
                    if sh == 0:
                        wk8.append(load_w_chunk(wk, i, "wk"))
                    xc = load_x(xkT, i, sh * SH)
                    for j in range(NC):
                        nc.tensor.matmul(
                            ps_k[j // 2][:, (j % 2) * SH:(j % 2 + 1) * SH],
                            wk8[i][:, j * P:(j + 1) * P], xc,
                            start=(i == 0), stop=(i == NC - 1),
                        )
                for j in range(NC):
                    eng = nc.scalar.copy if j % 2 == 0 else nc.vector.tensor_copy
                    eng(
                        kT[:, j, sh * SH:(sh + 1) * SH],
                        ps_k[j // 2][:, (j % 2) * SH:(j % 2 + 1) * SH],
                    )

            # ---------------- V projection ----------------
            # v[sk, d]; 2 passes over sk-chunk halves, each streams half of xv once
            wv8 = []
            for scg in range(2):
                ps_v = [ps_tile(f"psv{t}") for t in range(4)]
                for i in range(NC):
                    if scg == 0:
                        wv8.append(load_w_chunk(wv, i, "wv"))
                    xc = load_x(xvT, i, scg * SH)
                    for sl in range(4):
                        for dh in range(2):
                            nc.tensor.matmul(
                                ps_v[sl][:, dh * SH:(dh + 1) * SH],
                                xc[:, sl * P:(sl + 1) * P],
                                wv8[i][:, dh * SH:(dh + 1) * SH],
                                start=(i == 0), stop=(i == NC - 1),
                            )
                for sl in range(4):
                    sc = scg * 4 + sl
                    for dh in range(2):
                        eng = nc.scalar.copy if dh == 0 else nc.vector.tensor_copy
                        eng(
                            vt[:, sc, dh * 8:(dh + 1) * 8, :DK],
                            ps_v[sl][:, dh * SH:(dh + 1) * SH]
                            .rearrange("p (h d) -> p h d", d=DK),
                        )

            # ---------------- attention, head by head ----------------
            wo8 = load_w(wo, "wo")   # prefetch during attention
            heat = ps_tile("heat")

            def heater():
                # keep the PE busy through ACT-gated gaps so HAM stays warm
                nc.tensor.matmul(
                    heat[:1, :SH], qT[:, 0, :1], qT[:, 0, :],
                    start=True, stop=True,
                )

            for h in range(H):
                jc, base = h // 2, 64 * (h % 2)
                expt = expp.tile([P, NC, SH], f32r, tag="expt", name="expt")
                for cp in range(4):
                    ps = ps_tile("scps")
                    for k in range(2):
                        c = 2 * cp + k
                        nc.tensor.matmul(
                            ps[:, k * SH:(k + 1) * SH],
                            kT[base:base + DK, jc, c * P:(c + 1) * P],
                            qT[base:base + DK, jc, :],
                            start=True, stop=True,
                        )
                    nc.scalar.activation(
                        out=expt[:, 2 * cp:2 * cp + 2, :],
                        in_=ps.rearrange("p (a b) -> p a b", a=2),
                        func=mybir.ActivationFunctionType.Exp,
                        scale=1.0 / np.sqrt(np.float32(DK)),
                    )
                pv = ps_tile("pv")
                for c in range(NC):
                    nc.tensor.matmul(
                        pv[:DK + 1, :SH], vt[:, c, h, :], expt[:, c, :],
                        start=(c == 0), stop=(c == NC - 1),
                    )
                heater()
                sums_raw = small.tile([1, SH], f32, tag="sums_raw", name="sums_raw")
                nc.vector.tensor_copy(sums_raw, pv[DK:DK + 1, :SH])
                sums = small.tile([1, SH], f32, tag="sums", name="sums")
                nc.vector.reciprocal_approx_fast(sums, sums_raw)
                rbc = small.tile([DK, SH], f32, tag="rbc", name="rbc")
                nc.gpsimd.partition_broadcast(rbc, sums)
                nc.vector.tensor_mul(
                    out=xT[base:base + DK, jc, :], in0=pv[:DK, :SH], in1=rbc
                )

            # ---------------- output projection + residual + LN ----------------
            for i, t in enumerate((gamma, beta)):
                nc.sync.dma_start(
                    gb[:, i, :], bass.AP(tensor=t, offset=0, ap=[[0, P], [1, D]])
                )
            for sc in range(4):
                ps_osc = ps_tile(f"pso{sc}")
                for dc in range(NC):
                    for eh in range(2):
                        nc.tensor.matmul(
                            ps_osc[:, eh * SH:(eh + 1) * SH],
                            xT[:, dc, sc * P:(sc + 1) * P],
                            wo8[dc][:, eh * SH:(eh + 1) * SH],
                            start=(dc == 0), stop=(dc == NC - 1),
                        )
                xl = lnp.tile([P, D], f32, tag="xln", name="xl")
                for eh in range(2):
                    rc = xpool.tile([P, SH], f32, tag="x", name="rc")
                    for q in range(2):
                        nc.sync.dma_start(
                            rc[:, q * 256:(q + 1) * 256],
                            resid[sc * P:(sc + 1) * P,
                                  eh * SH + q * 256:eh * SH + (q + 1) * 256],
                        )
                    nc.vector.tensor_add(
                        out=xl[:, eh * SH:(eh + 1) * SH],
                        in0=ps_osc[:, eh * SH:(eh + 1) * SH], in1=rc,
                    )
                stats = small.tile([P, 2, nc.vector.BN_STATS_DIM], f32, tag="stats",
                                   name="stats")
                for i in range(2):
                    nc.vector.bn_stats(stats[:, i, :], xl[:, i * SH:(i + 1) * SH])
                mv = small.tile([P, nc.vector.BN_AGGR_DIM], f32, tag="mv", name="mv")
                nc.vector.bn_aggr(mv, stats)
                std = small.tile([P, 1], f32, tag="std", name="std")
                nc.scalar.activation(
                    out=std, in_=mv[:, 1:2],
                    func=mybir.ActivationFunctionType.Sqrt,
                    bias=eps_t, scale=1.0,
                )
                rstd = small.tile([P, 1], f32, tag="rstd", name="rstd")
                nc.vector.reciprocal_approx_fast(rstd, std)
                nc.vector.tensor_scalar(
                    out=xl, in0=xl, scalar1=mv[:, 0:1], scalar2=rstd,
                    op0=mybir.AluOpType.subtract, op1=mybir.AluOpType.mult,
                )
                nc.vector.tensor_mul(out=xl, in0=xl, in1=gb[:, 0, :])
                nc.vector.tensor_add(out=xl, in0=xl, in1=gb[:, 1, :])
                for q in range(2):
                    nc.sync.dma_start(
                        out[sc * P:(sc + 1) * P, q * SH:(q + 1) * SH],
                        xl[:, q * SH:(q + 1) * SH],
                    )

    nc.compile()
    return nc


def kernel(query, key, value, Wq, Wk, Wv, Wo, ln_gamma, ln_beta):
    global LAST_EXEC_NS
    if "nc" not in _CACHE:
        _CACHE["nc"] = _build()
    nc = _CACHE["nc"]

    query = np.asarray(query, np.float32)
    key = np.asarray(key, np.float32)
    value = np.asarray(value, np.float32)
    wqT = np.ascontiguousarray(np.asarray(Wq, np.float32).T)
    wkT = np.ascontiguousarray(np.asarray(Wk, np.float32).T)
    wvT = np.ascontiguousarray(np.asarray(Wv, np.float32).T)
    woT = np.ascontiguousarray(np.asarray(Wo, np.float32).T)
    gamma = np.ascontiguousarray(np.asarray(ln_gamma, np.float32))
    beta = np.ascontiguousarray(np.asarray(ln_beta, np.float32))

    in_maps = []
    for core in range(NCORES):
        b, half = core // 2, core % 2
        sl = slice(half * SH, (half + 1) * SH)
        in_maps.append({
            "xqT": np.ascontiguousarray(query[b].T[:, sl]),
            "xkT": np.ascontiguousarray(key[b].T),
            "xvT": np.ascontiguousarray(value[b].T),
            "wq": wqT, "wk": wkT, "wv": wvT, "wo": woT,
            "resid": np.ascontiguousarray(query[b, sl]),
            "gamma": gamma, "beta": beta,
        })

    res = bass_utils.run_bass_kernel_spmd(
        nc, in_maps, core_ids=list(range(NCORES)), trace=TRACE
    )
    LAST_EXEC_NS = res.exec_time_ns

    out = np.empty((B, S, D), np.float32)
    for core in range(NCORES):
        b, half = core // 2, core % 2
        out[b, half * SH:(half + 1) * SH] = np.asarray(res.results[core]["out"])
    return out


# revision 31
# speedup vs baseline: 1.1650x; 1.1650x over previous
"""nn_MultiHeadAttention: fused MHA + residual + LayerNorm on 8 TRN2 NeuronCores.

Sharding: core = (batch b, query-half). Each core computes, for its batch:
  - Q projection for its 512 query rows, K/V projections for all 1024 keys
    (K/V work duplicated within a batch pair -> zero cross-core communication),
  - all 16 heads' attention for its query rows,
  - output projection + residual + LayerNorm for its rows.
Host concatenates the 8 [512, 1024] results into [4, 1024, 1024].

All matmuls run in float32r (TF32-like, 1 cycle/row at N=512; plain fp32 is
4x slower). Layouts are transposed so softmax sums come free as an extra
ones-column in the PV matmul:
  qT/kT = W.T @ x.T   [d_model_out, s]   (head h = partitions 64*(h%2) of chunk h//2)
  scoresT[sk, sq] = kT_h.T @ qT_h        (K=64 matmul)
  p = exp(scores/8)   (no max-subtraction: scores ~ N(0,1), exp is safe in fp32)
  xT[d(+1), sq] = [v | 1].T @ p          (row 64 = softmax denominators)

PSUM discipline: every accumulator is a [128, 1024] 2-bank tile from one
4-slot pool (exactly 8 banks). The ACT-bound attention phase interleaves
dummy "heater" matmuls so the PE's HAM clock gate stays at 2.4 GHz.
"""
import numpy as np

import concourse.bass as bass
import concourse.mybir as mybir
import concourse.tile as tile
from concourse import bacc, bass_utils

B, S, D, H, DK = 4, 1024, 1024, 16, 64
P = 128
SH = S // 2           # query rows per core
NC = D // P           # 8 chunks of 128 along any d-dimension
NCORES = 8
EPS = 1e-6
f32 = mybir.dt.float32
f32r = mybir.dt.float32r

TRACE = False          # set by test.py to profile
LAST_EXEC_NS = None

_CACHE = {}


def _build():
    nc = bacc.Bacc("TRN2")
    xqT = nc.dram_tensor("xqT", [D, SH], f32, kind="ExternalInput")
    xkT = nc.dram_tensor("xkT", [D, S], f32, kind="ExternalInput")
    xvT = nc.dram_tensor("xvT", [D, S], f32, kind="ExternalInput")
    wq = nc.dram_tensor("wq", [D, D], f32, kind="ExternalInput")   # Wq.T  [in, out]
    wk = nc.dram_tensor("wk", [D, D], f32, kind="ExternalInput")
    wv = nc.dram_tensor("wv", [D, D], f32, kind="ExternalInput")
    wo = nc.dram_tensor("wo", [D, D], f32, kind="ExternalInput")   # Wo.T  [d, e]
    resid = nc.dram_tensor("resid", [SH, D], f32, kind="ExternalInput")
    gamma = nc.dram_tensor("gamma", [D], f32, kind="ExternalInput")
    beta = nc.dram_tensor("beta", [D], f32, kind="ExternalInput")
    out = nc.dram_tensor("out", [SH, D], f32, kind="ExternalOutput")

    with tile.TileContext(nc) as tc:
        with (
            tc.tile_pool(name="wpool", bufs=9) as wpool,
            tc.tile_pool(name="xpool", bufs=7) as xpool,
            tc.tile_pool(name="persist", bufs=1) as persist,
            tc.tile_pool(name="expp", bufs=2) as expp,
            tc.tile_pool(name="small", bufs=2) as small,
            tc.tile_pool(name="lnp", bufs=2) as lnp,
            tc.tile_pool(name="psum", bufs=4, space="PSUM") as psum,
        ):
            # ---------------- persistent tiles ----------------
            qT = persist.tile([P, NC, SH], f32r)     # [j, sq]
            kT = persist.tile([P, NC, S], f32r)      # [j, sk]
            vt = persist.tile([P, NC, H, DK + 1], f32r)  # [sk, (h, d|1)]
            xT = persist.tile([P, NC, SH], f32r)     # normalized attn out, [d, sq]
            gb = persist.tile([P, 2, D], f32)        # gamma/beta broadcast
            eps_t = persist.tile([P, 1], f32)

            nc.vector.memset(eps_t, EPS)
            nc.vector.memset(vt[:, :, :, DK:DK + 1].bitcast(f32), 1.0)  # ones col

            def load_w_chunk(w, i, nm):
                """One [128, 1024] weight chunk, 8x 64KB sub-DMAs."""
                wt = wpool.tile([P, D], f32r, tag="w", name=f"{nm}{i}")
                for q in range(4):
                    nc.sync.dma_start(
                        wt[:, q * 256:(q + 1) * 256],
                        w[i * P:(i + 1) * P, q * 256:(q + 1) * 256].bitcast(f32r),
                    )
                return wt

            def load_w(w, nm):
                return [load_w_chunk(w, i, nm) for i in range(NC)]

            def load_x(x, i, col0):
                """One [128, 512] half-chunk of a transposed input, 4x 64KB sub-DMAs."""
                xc = xpool.tile([P, SH], f32r, tag="x", name="xc")
                for q in range(2):
                    nc.sync.dma_start(
                        xc[:, q * 256:(q + 1) * 256],
                        x[i * P:(i + 1) * P, col0 + q * 256:col0 + (q + 1) * 256]
                        .bitcast(f32r),
                    )
                return xc

            def ps_tile(nm):
                return psum.tile([P, 2 * SH], f32, tag="mm", name=nm)

            # ---------------- Q projection ----------------
            # qT[j, sq] = sum_i Wq.T[i, j] * xqT[i, sq]
            ps_q = [ps_tile(f"psq{t}") for t in range(4)]
            wq8 = []
            for i in range(NC):
                wq8.append(load_w_chunk(wq, i, "wq"))
                xc = load_x(xqT, i, 0)
                for j in range(NC):
                    nc.tensor.matmul(
                        ps_q[j // 2][:, (j % 2) * SH:(j % 2 + 1) * SH],
                        wq8[i][:, j * P:(j + 1) * P], xc,
                        start=(i == 0), stop=(i == NC - 1),
                    )
            for t in range(4):
                nc.scalar.copy(qT[:, 2 * t, :], ps_q[t][:, :SH])
                nc.vector.tensor_copy(qT[:, 2 * t + 1, :], ps_q[t][:, SH:])

            # ---------------- K projection ----------------
            # kT[j, sk]; 2 passes over sk-halves, each streams half of xk once
            wk8 = []
            wv8 = []
            for sh in range(2):
                ps_k = [ps_tile(f"psk{t}") for t in range(4)]
                for i in range(NC):
                    if sh == 0:
                        wk8.append(load_w_chunk(wk, i, "wk"))
                    else:
                        wv8.append(load_w_chunk(wv, i, "wv"))
                    xc = load_x(xkT, i, sh * SH)
                    for j in range(NC):
                        nc.tensor.matmul(
                            ps_k[j // 2][:, (j % 2) * SH:(j % 2 + 1) * SH],
                            wk8[i][:, j * P:(j + 1) * P], xc,
                            start=(i == 0), stop=(i == NC - 1),
                        )
                for j in range(NC):
                    eng = nc.scalar.copy if j % 2 == 0 else nc.vector.tensor_copy
                    eng(
                        kT[:, j, sh * SH:(sh + 1) * SH],
                        ps_k[j // 2][:, (j % 2) * SH:(j % 2 + 1) * SH],
                    )

            # ---------------- V projection ----------------
            # v[sk, d]; 2 passes over sk-chunk halves, each streams half of xv once
            # (wv8 chunks were prefetched during the DMA-light K pass 1)
            for scg in range(2):
                ps_v = [ps_tile(f"psv{t}") for t in range(4)]
                for i in range(NC):
                    xc = load_x(xvT, i, scg * SH)
                    for sl in range(4):
                        for dh in range(2):
                            nc.tensor.matmul(
                                ps_v[sl][:, dh * SH:(dh + 1) * SH],
                                xc[:, sl * P:(sl + 1) * P],
                                wv8[i][:, dh * SH:(dh + 1) * SH],
                                start=(i == 0), stop=(i == NC - 1),
                            )
                for sl in range(4):
                    sc = scg * 4 + sl
                    for dh in range(2):
                        eng = nc.scalar.copy if dh == 0 else nc.vector.tensor_copy
                        eng(
                            vt[:, sc, dh * 8:(dh + 1) * 8, :DK],
                            ps_v[sl][:, dh * SH:(dh + 1) * SH]
                            .rearrange("p (h d) -> p h d", d=DK),
                        )

            # ---------------- attention, head by head ----------------
            wo8 = load_w(wo, "wo")   # prefetch during attention
            heat = ps_tile("heat")

            def heater():
                # keep the PE busy through ACT-gated gaps so HAM stays warm
                nc.tensor.matmul(
                    heat[:1, :SH], qT[:, 0, :1], qT[:, 0, :],
                    start=True, stop=True,
                )

            for h in range(H):
                jc, base = h // 2, 64 * (h % 2)
                expt = expp.tile([P, NC, SH], f32r, tag="expt", name="expt")
                for cp in range(4):
                    ps = ps_tile("scps")
                    for k in range(2):
                        c = 2 * cp + k
                        nc.tensor.matmul(
                            ps[:, k * SH:(k + 1) * SH],
                            kT[base:base + DK, jc, c * P:(c + 1) * P],
                            qT[base:base + DK, jc, :],
                            start=True, stop=True,
                        )
                    nc.scalar.activation(
                        out=expt[:, 2 * cp:2 * cp + 2, :],
                        in_=ps.rearrange("p (a b) -> p a b", a=2),
                        func=mybir.ActivationFunctionType.Exp,
                        scale=1.0 / np.sqrt(np.float32(DK)),
                    )
                pv = ps_tile("pv")
                for c in range(NC):
                    nc.tensor.matmul(
                        pv[:DK + 1, :SH], vt[:, c, h, :], expt[:, c, :],
                        start=(c == 0), stop=(c == NC - 1),
                    )
                heater()
                sums_raw = small.tile([1, SH], f32, tag="sums_raw", name="sums_raw")
                nc.vector.tensor_copy(sums_raw, pv[DK:DK + 1, :SH])
                sums = small.tile([1, SH], f32, tag="sums", name="sums")
                nc.vector.reciprocal_approx_fast(sums, sums_raw)
                rbc = small.tile([DK, SH], f32, tag="rbc", name="rbc")
                nc.gpsimd.partition_broadcast(rbc, sums)
                nc.vector.tensor_mul(
                    out=xT[base:base + DK, jc, :], in0=pv[:DK, :SH], in1=rbc
                )

            # ---------------- output projection + residual + LN ----------------
            for i, t in enumerate((gamma, beta)):
                nc.sync.dma_start(
                    gb[:, i, :], bass.AP(tensor=t, offset=0, ap=[[0, P], [1, D]])
                )
            for sc in range(4):
                ps_osc = ps_tile(f"pso{sc}")
                for dc in range(NC):
                    for eh in range(2):
                        nc.tensor.matmul(
                            ps_osc[:, eh * SH:(eh + 1) * SH],
                            xT[:, dc, sc * P:(sc + 1) * P],
                            wo8[dc][:, eh * SH:(eh + 1) * SH],
                            start=(dc == 0), stop=(dc == NC - 1),
                        )
                xl = lnp.tile([P, D], f32, tag="xln", name="xl")
                for eh in range(2):
                    rc = xpool.tile([P, SH], f32, tag="x", name="rc")
                    for q in range(2):
                        nc.sync.dma_start(
                            rc[:, q * 256:(q + 1) * 256],
                            resid[sc * P:(sc + 1) * P,
                                  eh * SH + q * 256:eh * SH + (q + 1) * 256],
                        )
                    nc.vector.tensor_add(
                        out=xl[:, eh * SH:(eh + 1) * SH],
                        in0=ps_osc[:, eh * SH:(eh + 1) * SH], in1=rc,
                    )
                stats = small.tile([P, 2, nc.vector.BN_STATS_DIM], f32, tag="stats",
                                   name="stats")
                for i in range(2):
                    nc.vector.bn_stats(stats[:, i, :], xl[:, i * SH:(i + 1) * SH])
                mv = small.tile([P, nc.vector.BN_AGGR_DIM], f32, tag="mv", name="mv")
                nc.vector.bn_aggr(mv, stats)
                std = small.tile([P, 1], f32, tag="std", name="std")
                nc.scalar.activation(
                    out=std, in_=mv[:, 1:2],
                    func=mybir.ActivationFunctionType.Sqrt,
                    bias=eps_t, scale=1.0,
                )
                rstd = small.tile([P, 1], f32, tag="rstd", name="rstd")
                nc.vector.reciprocal_approx_fast(rstd, std)
                nc.vector.tensor_scalar(
                    out=xl, in0=xl, scalar1=mv[:, 0:1], scalar2=rstd,
                    op0=mybir.AluOpType.subtract, op1=mybir.AluOpType.mult,
                )
                nc.vector.tensor_mul(out=xl, in0=xl, in1=gb[:, 0, :])
                nc.vector.tensor_add(out=xl, in0=xl, in1=gb[:, 1, :])
                for q in range(2):
                    nc.sync.dma_start(
                        out[sc * P:(sc + 1) * P, q * SH:(q + 1) * SH],
                        xl[:, q * SH:(q + 1) * SH],
                    )

    nc.compile()
    return nc


def kernel(query, key, value, Wq, Wk, Wv, Wo, ln_gamma, ln_beta):
    global LAST_EXEC_NS
    if "nc" not in _CACHE:
        _CACHE["nc"] = _build()
    nc = _CACHE["nc"]

    query = np.asarray(query, np.float32)
    key = np.asarray(key, np.float32)
    value = np.asarray(value, np.float32)
    wqT = np.ascontiguousarray(np.asarray(Wq, np.float32).T)
    wkT = np.ascontiguousarray(np.asarray(Wk, np.float32).T)
    wvT = np.ascontiguousarray(np.asarray(Wv, np.float32).T)
    woT = np.ascontiguousarray(np.asarray(Wo, np.float32).T)
    gamma = np.ascontiguousarray(np.asarray(ln_gamma, np.float32))
    beta = np.ascontiguousarray(np.asarray(ln_beta, np.float32))

    in_maps = []
    for core in range(NCORES):
        b, half = core // 2, core % 2
        sl = slice(half * SH, (half + 1) * SH)
        in_maps.append({
            "xqT": np.ascontiguousarray(query[b].T[:, sl]),
            "xkT": np.ascontiguousarray(key[b].T),
            "xvT": np.ascontiguousarray(value[b].T),
            "wq": wqT, "wk": wkT, "wv": wvT, "wo": woT,
            "resid": np.ascontiguousarray(query[b, sl]),
            "gamma": gamma, "beta": beta,
        })

    res = bass_utils.run_bass_kernel_spmd(
        nc, in_maps, core_ids=list(range(NCORES)), trace=TRACE
    )
    LAST_EXEC_NS = res.exec_time_ns

    out = np.empty((B, S, D), np.float32)
    for core in range(NCORES):
        b, half = core // 2, core % 2
        out[b, half * SH:(half + 1) * SH] = np.asarray(res.results[core]["out"])
    return out


# revision 32
# speedup vs baseline: 1.1858x; 1.0178x over previous
"""nn_MultiHeadAttention: fused MHA + residual + LayerNorm on 8 TRN2 NeuronCores.

Sharding: core = (batch b, query-half). Each core computes, for its batch:
  - Q projection for its 512 query rows, K/V projections for all 1024 keys
    (K/V work duplicated within a batch pair -> zero cross-core communication),
  - all 16 heads' attention for its query rows,
  - output projection + residual + LayerNorm for its rows.
Host concatenates the 8 [512, 1024] results into [4, 1024, 1024].

All matmuls run in float32r (TF32-like, 1 cycle/row at N=512; plain fp32 is
4x slower). Layouts are transposed so softmax sums come free as an extra
ones-column in the PV matmul:
  qT/kT = W.T @ x.T   [d_model_out, s]   (head h = partitions 64*(h%2) of chunk h//2)
  scoresT[sk, sq] = kT_h.T @ qT_h        (K=64 matmul)
  p = exp(scores/8)   (no max-subtraction: scores ~ N(0,1), exp is safe in fp32)
  xT[d(+1), sq] = [v | 1].T @ p          (row 64 = softmax denominators)

PSUM discipline: every accumulator is a [128, 1024] 2-bank tile from one
4-slot pool (exactly 8 banks). The ACT-bound attention phase interleaves
dummy "heater" matmuls so the PE's HAM clock gate stays at 2.4 GHz.
"""
import numpy as np

import concourse.bass as bass
import concourse.mybir as mybir
import concourse.tile as tile
from concourse import bacc, bass_utils

B, S, D, H, DK = 4, 1024, 1024, 16, 64
P = 128
SH = S // 2           # query rows per core
NC = D // P           # 8 chunks of 128 along any d-dimension
NCORES = 8
EPS = 1e-6
f32 = mybir.dt.float32
f32r = mybir.dt.float32r

TRACE = False          # set by test.py to profile
LAST_EXEC_NS = None

_CACHE = {}


def _build():
    nc = bacc.Bacc("TRN2")
    xqT = nc.dram_tensor("xqT", [D, SH], f32, kind="ExternalInput")
    xkT = nc.dram_tensor("xkT", [D, S], f32, kind="ExternalInput")
    xvT = nc.dram_tensor("xvT", [D, S], f32, kind="ExternalInput")
    wq = nc.dram_tensor("wq", [D, D], f32, kind="ExternalInput")   # Wq.T  [in, out]
    wk = nc.dram_tensor("wk", [D, D], f32, kind="ExternalInput")
    wv = nc.dram_tensor("wv", [D, D], f32, kind="ExternalInput")
    wo = nc.dram_tensor("wo", [D, D], f32, kind="ExternalInput")   # Wo.T  [d, e]
    resid = nc.dram_tensor("resid", [SH, D], f32, kind="ExternalInput")
    gamma = nc.dram_tensor("gamma", [D], f32, kind="ExternalInput")
    beta = nc.dram_tensor("beta", [D], f32, kind="ExternalInput")
    out = nc.dram_tensor("out", [SH, D], f32, kind="ExternalOutput")

    with tile.TileContext(nc) as tc:
        with (
            tc.tile_pool(name="wpool", bufs=9) as wpool,
            tc.tile_pool(name="xpool", bufs=7) as xpool,
            tc.tile_pool(name="persist", bufs=1) as persist,
            tc.tile_pool(name="expp", bufs=2) as expp,
            tc.tile_pool(name="small", bufs=2) as small,
            tc.tile_pool(name="lnp", bufs=2) as lnp,
            tc.tile_pool(name="psum", bufs=4, space="PSUM") as psum,
        ):
            # ---------------- persistent tiles ----------------
            qT = persist.tile([P, NC, SH], f32r)     # [j, sq]
            kT = persist.tile([P, NC, S], f32r)      # [j, sk]
            vt = persist.tile([P, NC, H, DK + 1], f32r)  # [sk, (h, d|1)]
            xT = persist.tile([P, NC, SH], f32r)     # normalized attn out, [d, sq]
            gb = persist.tile([P, 2, D], f32)        # gamma/beta broadcast
            eps_t = persist.tile([P, 1], f32)

            nc.vector.memset(eps_t, EPS)
            nc.vector.memset(vt[:, :, :, DK:DK + 1].bitcast(f32), 1.0)  # ones col

            def load_w_chunk(w, i, nm):
                """One [128, 1024] weight chunk, 8x 64KB sub-DMAs."""
                wt = wpool.tile([P, D], f32r, tag="w", name=f"{nm}{i}")
                for q in range(4):
                    nc.sync.dma_start(
                        wt[:, q * 256:(q + 1) * 256],
                        w[i * P:(i + 1) * P, q * 256:(q + 1) * 256].bitcast(f32r),
                    )
                return wt

            def load_w(w, nm):
                return [load_w_chunk(w, i, nm) for i in range(NC)]

            def load_x(x, i, col0):
                """One [128, 512] half-chunk of a transposed input, 4x 64KB sub-DMAs."""
                xc = xpool.tile([P, SH], f32r, tag="x", name="xc")
                for q in range(2):
                    nc.sync.dma_start(
                        xc[:, q * 256:(q + 1) * 256],
                        x[i * P:(i + 1) * P, col0 + q * 256:col0 + (q + 1) * 256]
                        .bitcast(f32r),
                    )
                return xc

            def ps_tile(nm):
                return psum.tile([P, 2 * SH], f32, tag="mm", name=nm)

            # ---------------- Q projection ----------------
            # qT[j, sq] = sum_i Wq.T[i, j] * xqT[i, sq]
            ps_q = [ps_tile(f"psq{t}") for t in range(4)]
            wq8 = []
            for i in range(NC):
                wq8.append(load_w_chunk(wq, i, "wq"))
                xc = load_x(xqT, i, 0)
                for j in range(NC):
                    nc.tensor.matmul(
                        ps_q[j // 2][:, (j % 2) * SH:(j % 2 + 1) * SH],
                        wq8[i][:, j * P:(j + 1) * P], xc,
                        start=(i == 0), stop=(i == NC - 1),
                    )
            for t in range(4):
                nc.scalar.copy(qT[:, 2 * t, :], ps_q[t][:, :SH])
                nc.vector.tensor_copy(qT[:, 2 * t + 1, :], ps_q[t][:, SH:])

            # ---------------- K projection ----------------
            # kT[j, sk]; 2 passes over sk-halves, each streams half of xk once
            wk8 = []
            wv8 = []
            for sh in range(2):
                ps_k = [ps_tile(f"psk{t}") for t in range(4)]
                for i in range(NC):
                    if sh == 0:
                        wk8.append(load_w_chunk(wk, i, "wk"))
                    xc = load_x(xkT, i, sh * SH)
                    for j in range(NC):
                        nc.tensor.matmul(
                            ps_k[j // 2][:, (j % 2) * SH:(j % 2 + 1) * SH],
                            wk8[i][:, j * P:(j + 1) * P], xc,
                            start=(i == 0), stop=(i == NC - 1),
                        )
                for j in range(NC):
                    eng = nc.scalar.copy if j % 2 == 0 else nc.vector.tensor_copy
                    eng(
                        kT[:, j, sh * SH:(sh + 1) * SH],
                        ps_k[j // 2][:, (j % 2) * SH:(j % 2 + 1) * SH],
                    )

            # ---------------- V projection ----------------
            # v[sk, d]; 2 passes over sk-chunk halves, each streams half of xv once
            for scg in range(2):
                ps_v = [ps_tile(f"psv{t}") for t in range(4)]
                for i in range(NC):
                    if scg == 0:
                        wv8.append(load_w_chunk(wv, i, "wv"))
                    xc = load_x(xvT, i, scg * SH)
                    for sl in range(4):
                        for dh in range(2):
                            nc.tensor.matmul(
                                ps_v[sl][:, dh * SH:(dh + 1) * SH],
                                xc[:, sl * P:(sl + 1) * P],
                                wv8[i][:, dh * SH:(dh + 1) * SH],
                                start=(i == 0), stop=(i == NC - 1),
                            )
                for sl in range(4):
                    sc = scg * 4 + sl
                    for dh in range(2):
                        eng = nc.scalar.copy if dh == 0 else nc.vector.tensor_copy
                        eng(
                            vt[:, sc, dh * 8:(dh + 1) * 8, :DK],
                            ps_v[sl][:, dh * SH:(dh + 1) * SH]
                            .rearrange("p (h d) -> p h d", d=DK),
                        )

            # ---------------- attention, head by head ----------------
            wo8 = load_w(wo, "wo")   # prefetch during attention
            heat = ps_tile("heat")

            def heater():
                # keep the PE busy through ACT-gated gaps so HAM stays warm
                nc.tensor.matmul(
                    heat[:1, :SH], qT[:, 0, :1], qT[:, 0, :],
                    start=True, stop=True,
                )

            for h in range(H):
                jc, base = h // 2, 64 * (h % 2)
                expt = expp.tile([P, NC, SH], f32r, tag="expt", name="expt")
                for cp in range(4):
                    ps = ps_tile("scps")
                    for k in range(2):
                        c = 2 * cp + k
                        nc.tensor.matmul(
                            ps[:, k * SH:(k + 1) * SH],
                            kT[base:base + DK, jc, c * P:(c + 1) * P],
                            qT[base:base + DK, jc, :],
                            start=True, stop=True,
                        )
                    nc.scalar.activation(
                        out=expt[:, 2 * cp:2 * cp + 2, :],
                        in_=ps.rearrange("p (a b) -> p a b", a=2),
                        func=mybir.ActivationFunctionType.Exp,
                        scale=1.0 / np.sqrt(np.float32(DK)),
                    )
                pv = ps_tile("pv")
                for c in range(NC):
                    nc.tensor.matmul(
                        pv[:DK + 1, :SH], vt[:, c, h, :], expt[:, c, :],
                        start=(c == 0), stop=(c == NC - 1),
                    )
                heater()
                sums_raw = small.tile([1, SH], f32, tag="sums_raw", name="sums_raw")
                nc.vector.tensor_copy(sums_raw, pv[DK:DK + 1, :SH])
                sums = small.tile([1, SH], f32, tag="sums", name="sums")
                nc.vector.reciprocal_approx_fast(sums, sums_raw)
                rbc = small.tile([DK, SH], f32, tag="rbc", name="rbc")
                nc.gpsimd.partition_broadcast(rbc, sums)
                nc.vector.tensor_mul(
                    out=xT[base:base + DK, jc, :], in0=pv[:DK, :SH], in1=rbc
                )

            # ---------------- output projection + residual + LN ----------------
            for i, t in enumerate((gamma, beta)):
                nc.sync.dma_start(
                    gb[:, i, :], bass.AP(tensor=t, offset=0, ap=[[0, P], [1, D]])
                )
            for sc in range(4):
                ps_osc = ps_tile(f"pso{sc}")
                for dc in range(NC):
                    for eh in range(2):
                        nc.tensor.matmul(
                            ps_osc[:, eh * SH:(eh + 1) * SH],
                            xT[:, dc, sc * P:(sc + 1) * P],
                            wo8[dc][:, eh * SH:(eh + 1) * SH],
                            start=(dc == 0), stop=(dc == NC - 1),
                        )
                xl = lnp.tile([P, D], f32, tag="xln", name="xl")
                for eh in range(2):
                    rc = xpool.tile([P, SH], f32, tag="x", name="rc")
                    for q in range(2):
                        nc.sync.dma_start(
                            rc[:, q * 256:(q + 1) * 256],
                            resid[sc * P:(sc + 1) * P,
                                  eh * SH + q * 256:eh * SH + (q + 1) * 256],
                        )
                    nc.vector.tensor_add(
                        out=xl[:, eh * SH:(eh + 1) * SH],
                        in0=ps_osc[:, eh * SH:(eh + 1) * SH], in1=rc,
                    )
                stats = small.tile([P, 2, nc.vector.BN_STATS_DIM], f32, tag="stats",
                                   name="stats")
                for i in range(2):
                    nc.vector.bn_stats(stats[:, i, :], xl[:, i * SH:(i + 1) * SH])
                mv = small.tile([P, nc.vector.BN_AGGR_DIM], f32, tag="mv", name="mv")
                nc.vector.bn_aggr(mv, stats)
                std = small.tile([P, 1], f32, tag="std", name="std")
                nc.scalar.activation(
                    out=std, in_=mv[:, 1:2],
                    func=mybir.ActivationFunctionType.Sqrt,
                    bias=eps_t, scale=1.0,
                )
                rstd = small.tile([P, 1], f32, tag="rstd", name="rstd")
                nc.vector.reciprocal_approx_fast(rstd, std)
                nc.vector.tensor_scalar(
                    out=xl, in0=xl, scalar1=mv[:, 0:1], scalar2=rstd,
                    op0=mybir.AluOpType.subtract, op1=mybir.AluOpType.mult,
                )
                nc.vector.tensor_mul(out=xl, in0=xl, in1=gb[:, 0, :])
                nc.vector.tensor_add(out=xl, in0=xl, in1=gb[:, 1, :])
                for q in range(2):
                    nc.sync.dma_start(
                        out[sc * P:(sc + 1) * P, q * SH:(q + 1) * SH],
                        xl[:, q * SH:(q + 1) * SH],
                    )

    nc.compile()
    return nc


def kernel(query, key, value, Wq, Wk, Wv, Wo, ln_gamma, ln_beta):
    global LAST_EXEC_NS
    if "nc" not in _CACHE:
        _CACHE["nc"] = _build()
    nc = _CACHE["nc"]

    query = np.asarray(query, np.float32)
    key = np.asarray(key, np.float32)
    value = np.asarray(value, np.float32)
    wqT = np.ascontiguousarray(np.asarray(Wq, np.float32).T)
    wkT = np.ascontiguousarray(np.asarray(Wk, np.float32).T)
    wvT = np.ascontiguousarray(np.asarray(Wv, np.float32).T)
    woT = np.ascontiguousarray(np.asarray(Wo, np.float32).T)
    gamma = np.ascontiguousarray(np.asarray(ln_gamma, np.float32))
    beta = np.ascontiguousarray(np.asarray(ln_beta, np.float32))

    in_maps = []
    for core in range(NCORES):
        b, half = core // 2, core % 2
        sl = slice(half * SH, (half + 1) * SH)
        in_maps.append({
            "xqT": np.ascontiguousarray(query[b].T[:, sl]),
            "xkT": np.ascontiguousarray(key[b].T),
            "xvT": np.ascontiguousarray(value[b].T),
            "wq": wqT, "wk": wkT, "wv": wvT, "wo": woT,
            "resid": np.ascontiguousarray(query[b, sl]),
            "gamma": gamma, "beta": beta,
        })

    res = bass_utils.run_bass_kernel_spmd(
        nc, in_maps, core_ids=list(range(NCORES)), trace=TRACE
    )
    LAST_EXEC_NS = res.exec_time_ns

    out = np.empty((B, S, D), np.float32)
    for core in range(NCORES):
        b, half = core // 2, core % 2
        out[b, half * SH:(half + 1) * SH] = np.asarray(res.results[core]["out"])
    return out
